# revision 60
# baseline (speedup 1.0000x reference)
"""Trainium2 Bass kernel for nn_CaptioningTransformer.

Data-parallel over batch N=8 across the 8 NeuronCores (one caption per core).
Each core runs the full 2-layer decoder + the (512,512)@(512,32000) logits
projection for its caption.

Big GEMMs run in fp8 (e4m3) DoubleRow mode (2x PE throughput per pass):
weights are split hi/lo (hi = fp8(w*64), lo = fp8(w*64 - hi)) so the weight
side is bf16-accurate; the dominant logits GEMM additionally splits the
activation hi/lo and computes the 3 leading terms (hi*hi + lo*hi + hi*lo).
Scores/AV stay bf16. LayerNorm / softmax statistics / residual stream are
fp32.  Logits are written to DRAM in bf16 and upcast on the host.

Self-contained: hardcodes all shapes; takes FULL inputs, returns FULL output.
"""

import math
from contextlib import ExitStack

import ml_dtypes
import numpy as np

import concourse.bacc as bacc
import concourse.bass as bass
import concourse.tile as tile
from concourse import mybir
from concourse.bass_utils import run_bass_kernel_spmd
from concourse.masks import make_identity

# dims
N, T, D, W, H, V, L, FF = 8, 512, 1024, 512, 4, 32000, 2, 2048
P = 128
TC = T // P            # 4 token chunks
KC = W // P            # 4 feature chunks
DC = D // P            # 8 vis-feature chunks
FFC = FF // P          # 16 ffn chunks
HD = W // H            # 128 head dim (== P)
VG = 1000              # vocab columns per DMA group
NVG = V // VG          # 32 groups
SV = 500               # vocab columns per psum tile
NSV = VG // SV         # 2 subtiles per group
EPS = 1e-5
SCALE = 1.0 / math.sqrt(HD)
CPACK_COLS = 4 + DC + 4 * L + 4 * L + 4 * L + FFC * L + W * L

# fp8 scaling
SX = 8.0               # activations entering qkv/ffn1/logits
SW = 64.0              # all hi/lo weights
SH = 32.0              # ffn hidden
EXP_BIAS = -8.0 * math.log(2.0)   # exp() outputs scaled by 2^-8
QK_OSCALE = 1.0 / (SX * SW)
H_OSCALE = SH / (SX * SW)
F2_OSCALE = 1.0 / (SH * SW)
LG_OSCALE = 1.0 / (SX * SW)

F32 = mybir.dt.float32
BF16 = mybir.dt.bfloat16
F8 = mybir.dt.float8e4
I32 = mybir.dt.int32
AX = mybir.AxisListType
ALU = mybir.AluOpType
ACTF = mybir.ActivationFunctionType
DR = mybir.MatmulPerfMode.DoubleRow
BF16_NP = ml_dtypes.bfloat16
F8_NP = ml_dtypes.float8_e4m3


def _wrap_p(a, np_dtype):
    """[..., k*P, X] -> [..., P, k, X] (partition-major wrap of the -2 axis)."""
    a = np.asarray(a)
    lead = a.shape[:-2]
    k = a.shape[-2] // P
    x = a.shape[-1]
    a = a.reshape(*lead, k, P, x)
    a = np.moveaxis(a, -2, -3)  # [..., P, k, x]
    return np.ascontiguousarray(a.astype(np_dtype))


def _wrap_vec(v, np_dtype):
    """[..., k*P] -> [..., P, k]."""
    v = np.asarray(v)
    lead = v.shape[:-1]
    k = v.shape[-1] // P
    v = v.reshape(*lead, k, P)
    v = np.moveaxis(v, -1, -2)
    return np.ascontiguousarray(v.astype(np_dtype))


def _split_f8(a, s):
    """f32 array -> (hi, lo) fp8 pair at scale s (hi + lo ~= a * s)."""
    a = np.asarray(a, np.float32) * s
    hi = a.astype(F8_NP)
    lo = (a - hi.astype(np.float32)).astype(F8_NP)
    return hi, lo


def _build(row_biases_zero: bool, ln_trivial: bool, cbz: bool = True,
           stop_after: str | None = None):
    nc = bacc.Bacc(
        "TRN2", target_bir_lowering=False, debug=False, enable_asserts=False
    )

    def din(name, shape, dt):
        return nc.dram_tensor(name, list(shape), dt, kind="ExternalInput").ap()

    # ---- DRAM inputs (per core) ----
    x0_d = din("x0", [P, TC, W], BF16)            # emb[captions] + pe (host)
    xt0_d = din("xt0", [P, KC, T], F8)            # SX * x0 transposed (host)
    visw_d = din("visw", [P, DC, W], BF16)
    # packed f32 consts: visb(4) feat(8) sabq(2*4) sabk(2*4) cabv(2*4)
    # ff1b(2*16, pre-scaled by SH) then cabo rows (row 0 only, 2*512)
    cpack_d = din("cpack", [P, CPACK_COLS], F32)
    sa_d = din("sa", [L, P, 3, 2, KC, W], F8)     # q,k,v x hi/lo
    sao_d = din("sao", [L, P, KC, W], BF16)       # out-proj stays bf16
    ca_d = din("ca", [L, P, 2, KC, W], BF16)      # wv,wo packed
    ff_d = din("ff", [L, P, 2, 2, KC * FF], F8)   # (f1,f2) x hi/lo flat
    outw_d = din("outw", [NVG, P, 2, KC, VG], F8)  # hi/lo per vocab group
    if not row_biases_zero:
        sabv_d = din("sabv", [L, 1, W], BF16)     # pre-scaled by SX*SW
        sabo_d = din("sabo", [L, 1, W], BF16)
        ff2b_d = din("ff2b", [L, 1, W], BF16)     # pre-scaled by SH*SW
        outb_d = din("outb", [1, V], BF16)        # pre-scaled by SX*SW
    if not ln_trivial:
        lnw_d = [din(f"ln{i}w", [L, 1, W], F32) for i in (1, 2, 3)]
        lnb_d = [din(f"ln{i}b", [L, 1, W], F32) for i in (1, 2, 3)]

    out_d = nc.dram_tensor("logits", [P, TC, V], BF16, kind="ExternalOutput").ap()

    with tile.TileContext(nc) as tc, ExitStack() as ctx:
        consts = ctx.enter_context(tc.tile_pool(name="consts", bufs=1))
        xpool = ctx.enter_context(tc.tile_pool(name="xpool", bufs=1))
        wpool = ctx.enter_context(tc.tile_pool(name="wpool", bufs=1))
        work = ctx.enter_context(tc.tile_pool(name="work", bufs=1))
        hot = ctx.enter_context(tc.tile_pool(name="hot", bufs=3))
        wlogp = ctx.enter_context(tc.tile_pool(name="wlogp", bufs=5))
        ostp = ctx.enter_context(tc.tile_pool(name="ostp", bufs=8))
        psA = ctx.enter_context(tc.tile_pool(name="psA", bufs=2, space="PSUM"))
        psS = ctx.enter_context(tc.tile_pool(name="psS", bufs=3, space="PSUM"))
        psT = ctx.enter_context(tc.tile_pool(name="psT", bufs=3, space="PSUM"))

        # engine rotation for copies / scaled copies
        _rr = [0]

        def copy_scaled(out, in_, scale):
            # Pool/GPSIMD cannot access PSUM on hw: rotate DVE/Act only
            i = _rr[0] % 2
            _rr[0] += 1
            if i == 0:
                nc.vector.tensor_scalar(out, in_, scale, None, ALU.mult)
            else:
                nc.scalar.activation(out, in_, ACTF.Copy, bias=0.0, scale=scale)

        # ---- constants ----
        ident_f32 = consts.tile([P, P], F32)
        make_identity(nc, ident_f32[:])
        # causal mask as a rank-P additive matmul: (maskA^T @ maskB)[i,j]
        # = -1e9 * max(0, i - j)  (0 on/above the diagonal)
        maskA = consts.tile([P, P], BF16)
        nc.vector.memset(maskA[:], 1.0)
        nc.gpsimd.affine_select(
            out=maskA[:], in_=maskA[:], compare_op=ALU.is_gt,
            fill=0.0, base=0, pattern=[[1, P]], channel_multiplier=-1,
        )
        maskB = consts.tile([P, P], BF16)
        nc.vector.memset(maskB[:], -1e9)
        nc.gpsimd.affine_select(
            out=maskB[:], in_=maskB[:], compare_op=ALU.is_ge,
            fill=0.0, base=0, pattern=[[-1, P]], channel_multiplier=1,
        )
        ones_col_bf = consts.tile([P, 1], BF16)
        nc.vector.memset(ones_col_bf[:], 1.0)
        ones_bf = consts.tile([1, P], BF16)
        nc.vector.memset(ones_bf[:], 1.0)
        ones_f32 = consts.tile([1, P], F32)
        nc.vector.memset(ones_f32[:], 1.0)
        eps_sb = consts.tile([P, 1], F32)
        nc.vector.memset(eps_sb[:], EPS)
        expb_sb = consts.tile([P, 1], F32)
        nc.vector.memset(expb_sb[:], EXP_BIAS)

        cpack_sb = consts.tile([P, CPACK_COLS], F32)
        o = 0
        visb_sb = cpack_sb[:, o : o + KC]; o += KC
        feat_sb = cpack_sb[:, o : o + DC]; o += DC
        sabq_sb = [cpack_sb[:, o + 4 * l : o + 4 * (l + 1)] for l in range(L)]
        o += 4 * L
        sabk_sb = [cpack_sb[:, o + 4 * l : o + 4 * (l + 1)] for l in range(L)]
        o += 4 * L
        cabv_sb = [cpack_sb[:, o + 4 * l : o + 4 * (l + 1)] for l in range(L)]
        o += 4 * L
        ff1b_sb = [cpack_sb[:, o + FFC * l : o + FFC * (l + 1)] for l in range(L)]
        o += FFC * L
        cabo_sb = [cpack_sb[0:1, o + W * l : o + W * (l + 1)] for l in range(L)]
        o += W * L
        featb_sb = consts.tile([P, DC], BF16)

        def per_layer_rows(dram, nm, dt, shape):
            tiles = []
            for l in range(L):
                t = consts.tile(shape, dt, name=f"{nm}{l}")
                nc.sync.dma_start(t[:], dram[l])
                tiles.append(t)
            return tiles
        if not row_biases_zero:
            sabv_sb = per_layer_rows(sabv_d, "sabv", BF16, [1, W])
            sabo_sb = per_layer_rows(sabo_d, "sabo", BF16, [1, W])
            ff2b_sb = per_layer_rows(ff2b_d, "ff2b", BF16, [1, W])
            outb_sb = consts.tile([1, V], BF16)
            nc.sync.dma_start(outb_sb[:], outb_d[:])
        if not ln_trivial:
            lnw_bc = [[None] * L for _ in range(3)]
            lnb_bc = [[None] * L for _ in range(3)]
            for i in range(3):
                for l in range(L):
                    wt = consts.tile([P, W], F32, name=f"lnwbc{i}_{l}")
                    nc.gpsimd.dma_start(wt[:], lnw_d[i][l].to_broadcast([P, W]))
                    lnw_bc[i][l] = wt
                    bt = consts.tile([P, W], F32, name=f"lnbbc{i}_{l}")
                    nc.gpsimd.dma_start(bt[:], lnb_d[i][l].to_broadcast([P, W]))
                    lnb_bc[i][l] = bt

        # ---- residual stream: x0 = emb[captions] + pe computed on HOST;
        # also shipped pre-transposed+fp8-scaled as xt0 (layer-0 SA input) ----
        x_sb = xpool.tile([P, TC, W], F32)
        xt0_sb = work.tile([P, KC, T], F8, name="xt0_sb", tag="xT")
        nc.sync.dma_start(xt0_sb[:], xt0_d[:])

        # ---- layer-0 self-attention weights (before x0: first qk matmul
        # needs only xt0 + saq) ----
        sa0_sb = wpool.tile([P, 3, 2, KC, W], F8, name="sa_sb", tag="sa_sb")
        nc.sync.dma_start(sa0_sb[:, 0, 0], sa_d[0, :, 0, 0])  # q hi first
        nc.sync.dma_start(sa0_sb[:, 0, 1], sa_d[0, :, 0, 1])
        for mi in (1, 2):
            nc.sync.dma_start(sa0_sb[:, mi], sa_d[0, :, mi])
        x0_sb = work.tile([P, TC, W], BF16, name="x0_sb", tag="x0")
        nc.sync.dma_start(x0_sb[:], x0_d[:])
        # constants packed tensor arrives after the critical-path DMAs
        nc.sync.dma_start(cpack_sb[:], cpack_d[:])
        sao0_sb = wpool.tile([P, KC, W], BF16, name="sao_sb", tag="sao_sb")
        nc.sync.dma_start(sao0_sb[:], sao_d[0])

        # ---- vis projection weights ----
        visw_sb = work.tile([P, DC, W], BF16, name="visw_sb", tag="visw")
        nc.sync.dma_start(visw_sb[:], visw_d[:])
        cal_sb = wpool.tile([P, L, 2, KC, W], BF16, name="ca_sb", tag="ca_sb")
        for l in range(L):
            nc.sync.dma_start(cal_sb[:, l], ca_d[l])

        _stages = {
            "embed": 0, "memT": 1, "sa0": 2, "ca0": 3, "l0": 4, "l1": 5,
            "logits1": 6, None: 99,
        }
        srank = _stages[stop_after]

        memT_sb = consts.tile([P, KC], BF16)
        ca_bc = []

        def compute_mem_and_ca():
            """memT = (features @ vis_w + vis_b)^T and the x-independent
            cross-attention broadcast rows.  Emitted mid-layer-0 so these
            PE ops don't block the SA matmuls at the head of PE's queue."""
            nc.vector.tensor_copy(featb_sb[:], feat_sb)
            for o in range(KC):
                pm = psS.tile([P, 512], F32, name="psS", tag="psS")
                for ki in range(DC):
                    nc.tensor.matmul(
                        pm[:, :1],
                        lhsT=visw_sb[:, ki, o * P : (o + 1) * P],
                        rhs=featb_sb[:, ki : ki + 1],
                        start=(ki == 0),
                        stop=(ki == DC - 1),
                    )
                nc.scalar.activation(
                    memT_sb[:, o : o + 1], pm[:, :1], ACTF.Identity,
                    bias=visb_sb[:, o : o + 1], scale=1.0,
                )
            for l in range(L):
                cawv_sb, cawo_sb = cal_sb[:, l, 0], cal_sb[:, l, 1]
                vTca = hot.tile([P, KC], BF16, name="vTca", tag="vTca")
                for o in range(KC):
                    pm = psS.tile([P, 512], F32, name="psS", tag="psS")
                    for ki in range(KC):
                        nc.tensor.matmul(
                            pm[:, :1],
                            lhsT=cawv_sb[:, ki, o * P : (o + 1) * P],
                            rhs=memT_sb[:, ki : ki + 1],
                            start=(ki == 0),
                            stop=(ki == KC - 1),
                        )
                    nc.scalar.activation(
                        vTca[:, o : o + 1], pm[:, :1], ACTF.Identity,
                        bias=cabv_sb[l][:, o : o + 1], scale=1.0,
                    )
                pr = psS.tile([P, 512], F32, name="psS", tag="psS")
                for o in range(KC):
                    nc.tensor.matmul(
                        pr[:1, :],
                        lhsT=vTca[:, o : o + 1],
                        rhs=cawo_sb[:, o, :],
                        start=(o == 0),
                        stop=(o == KC - 1),
                    )
                ca_row = hot.tile([1, W], F32, name="ca_row", tag="ca_row")
                nc.vector.tensor_tensor(
                    ca_row[:], pr[:1, :], cabo_sb[l], op=ALU.add
                )
                pbc = psS.tile([P, 512], F32, name="psS", tag="psS")
                nc.tensor.matmul(
                    pbc[:], lhsT=ones_f32[:], rhs=ca_row[:], start=True, stop=True
                )
                cb = consts.tile([P, W], F32, name=f"ca_bc{l}")
                nc.scalar.copy(cb[:], pbc[:])
                ca_bc.append(cb)

        if srank in (1, 2):
            compute_mem_and_ca()

        def ln_chunk(ln_idx, l, c):
            """x_sb[:, c] <- LN(x_sb[:, c]) (free-axis stats)."""
            stats = hot.tile([P, 6], F32, name="lnstats", tag="lnstats")
            nc.vector.bn_stats(stats[:], x_sb[:, c, :])
            mv = hot.tile([P, 2], F32, name="lnmv", tag="lnmv")
            nc.vector.bn_aggr(mv[:], stats[:])
            std = hot.tile([P, 1], F32, name="lnstd", tag="lnstd")
            nc.scalar.activation(
                std[:], mv[:, 1:2], ACTF.Sqrt, bias=eps_sb[:], scale=1.0
            )
            rstd = hot.tile([P, 1], F32, name="lnrstd", tag="lnrstd")
            nc.vector.reciprocal(rstd[:], std[:])
            nmr = hot.tile([P, 1], F32, name="lnnmr", tag="lnnmr")
            nc.vector.scalar_tensor_tensor(
                nmr[:], mv[:, 0:1], -1.0, rstd[:],
                op0=ALU.mult, op1=ALU.mult,
            )
            nc.scalar.activation(
                x_sb[:, c, :], x_sb[:, c, :], ACTF.Identity,
                bias=nmr[:], scale=rstd[:],
            )
            if not ln_trivial:
                nc.vector.tensor_tensor(
                    x_sb[:, c, :], x_sb[:, c, :], lnw_bc[ln_idx][l][:],
                    op=ALU.mult,
                )
                nc.vector.tensor_tensor(
                    x_sb[:, c, :], x_sb[:, c, :], lnb_bc[ln_idx][l][:],
                    op=ALU.add,
                )

        def layer_norm(ln_idx, l):
            for c in range(TC):
                ln_chunk(ln_idx, l, c)

        lnsq_sb = work.tile([P, W], F32, name="lnsq", tag="lnsq")

        def ln_chunk_fast(c, sx):
            """LN with sum(x) from the producer's accum_out and sum(x^2)
            via an Act Square pass: frees DVE of bn_stats/bn_aggr."""
            sxx = hot.tile([P, 1], F32, name="lnsxx", tag="lnsxx")
            nc.scalar.activation(
                lnsq_sb[:], x_sb[:, c, :], ACTF.Square, bias=0.0, scale=1.0,
                accum_out=sxx[:],
            )
            m = hot.tile([P, 1], F32, name="lnm", tag="lnm")
            nc.vector.tensor_scalar(m[:], sx[:], 1.0 / W, None, ALU.mult)
            var = hot.tile([P, 1], F32, name="lnvar", tag="lnvar")
            msq = hot.tile([P, 1], F32, name="lnmsq", tag="lnmsq")
            nc.vector.tensor_tensor(msq[:], m[:], m[:], op=ALU.mult)
            nc.vector.scalar_tensor_tensor(
                var[:], sxx[:], 1.0 / W, msq[:], op0=ALU.mult, op1=ALU.subtract
            )
            std = hot.tile([P, 1], F32, name="lnstd", tag="lnstd")
            nc.scalar.activation(
                std[:], var[:], ACTF.Sqrt, bias=eps_sb[:], scale=1.0
            )
            rstd = hot.tile([P, 1], F32, name="lnrstd", tag="lnrstd")
            nc.vector.reciprocal(rstd[:], std[:])
            nmr = hot.tile([P, 1], F32, name="lnnmr", tag="lnnmr")
            nc.vector.scalar_tensor_tensor(
                nmr[:], m[:], -1.0, rstd[:], op0=ALU.mult, op1=ALU.mult,
            )
            nc.scalar.activation(
                x_sb[:, c, :], x_sb[:, c, :], ACTF.Identity,
                bias=nmr[:], scale=rstd[:],
            )

        def transpose_chunk(c, xt_hi, xt_lo=None):
            """xt_hi[p, o, c*P:+P] (fp8) <- SX * x_sb[t%P, c, o*P+p]."""
            for o in range(KC):
                pt = psT.tile([P, P], F32, name="ptr", tag="ptr")
                nc.tensor.transpose(
                    pt[:], x_sb[:, c, o * P : (o + 1) * P], ident_f32[:]
                )
                dst = xt_hi[:, o, c * P : (c + 1) * P]
                nc.scalar.activation(
                    dst, pt[:], ACTF.Copy, bias=0.0, scale=SX
                )
                if xt_lo is not None:
                    nc.vector.scalar_tensor_tensor(
                        xt_lo[:, o, c * P : (c + 1) * P], pt[:], SX, dst,
                        op0=ALU.mult, op1=ALU.subtract,
                    )

        def transpose_x_to(xt_hi, xt_lo=None):
            for c in range(TC):
                transpose_chunk(c, xt_hi, xt_lo)

        # ================= layers =================
        sal_next, sao_next = sa0_sb, sao0_sb
        for l in range(L if srank >= 2 else 0):
            # ---- self attention ----
            sal_sb, saol_sb = sal_next, sao_next
            # this layer's FFN weights stream during SA compute
            ffl_sb = wpool.tile([P, 2, 2, KC * FF], F8, name="ff_sb", tag="ff_sb")
            nc.sync.dma_start(ffl_sb[:], ff_d[l])
            # [P, 2(hi/lo), KC, W] each
            saq_sb, sak_sb, sav_sb = sal_sb[:, 0], sal_sb[:, 1], sal_sb[:, 2]

            if l == 0:
                xT = xt0_sb
            else:
                xT = work.tile([P, KC, T], F8, name="xT", tag="xT")
                transpose_x_to(xT)

            qT = work.tile([P, KC, T], BF16, name="qT", tag="qT")
            kT = work.tile([P, KC, T], BF16, name="kT", tag="kT")
            for dst, wsb, bsb in ((qT, saq_sb, sabq_sb[l]), (kT, sak_sb, sabk_sb[l])):
                for o in range(KC):
                    pq = psA.tile([P, 512], F32, name="psA", tag="psA")
                    first = True
                    for hl in range(2):
                        for k0 in range(0, KC, 2):
                            nc.tensor.matmul(
                                pq[:],
                                lhsT=wsb[:, hl, k0 : k0 + 2, o * P : (o + 1) * P],
                                rhs=xT[:, k0 : k0 + 2, :],
                                start=first,
                                stop=(hl == 1 and k0 == KC - 2),
                                perf_mode=DR,
                            )
                            first = False
                    if cbz:
                        # q/k biases zero: pure scale on DVE (Act is the SA
                        # window pacer with exp + yT copies)
                        nc.vector.tensor_scalar(
                            dst[:, o, :], pq[:], QK_OSCALE, None, ALU.mult
                        )
                    else:
                        nc.scalar.activation(
                            dst[:, o, :], pq[:], ACTF.Identity,
                            bias=bsb[:, o : o + 1], scale=QK_OSCALE,
                        )
            v_sb = work.tile([P, TC, W], BF16, name="v_sb", tag="v_sb")
            for c in range(TC):
                pv = psA.tile([P, 512], F32, name="psA", tag="psA")
                first = True
                if not row_biases_zero:
                    nc.tensor.matmul(
                        pv[:], lhsT=ones_bf[:], rhs=sabv_sb[l][:],
                        start=True, stop=False,
                    )
                    first = False
                for hl in range(2):
                    for k0 in range(0, KC, 2):
                        nc.tensor.matmul(
                            pv[:],
                            lhsT=xT[:, k0 : k0 + 2, c * P : (c + 1) * P],
                            rhs=sav_sb[:, hl, k0 : k0 + 2, :],
                            start=first,
                            stop=(hl == 1 and k0 == KC - 2),
                            perf_mode=DR,
                        )
                        first = False
                nc.vector.tensor_scalar(
                    v_sb[:, c, :], pv[:], QK_OSCALE, None, ALU.mult
                )

            yT = work.tile([P, H, T], BF16, name="yT", tag="yT")
            rinv_all = work.tile([P, H, TC], F32, name="rinv_all",
                                 tag="rinv_all", bufs=2)
            for h in range(H):
                # scores computed pre-transposed [tk, tq] (swap q/k roles), so
                # exp() writes the A@V operand directly -- no PE transposes.
                # Probs are UNNORMALIZED but scaled by 2^-8 (exp bias);
                # normalization applied per-head at the out-projection.
                AT = work.tile([P, TC, T], BF16, name="AT", tag="AT", bufs=3)
                for j in range(TC):
                    nv = T - j * P  # valid tq suffix for tk-chunk j
                    ps = psS.tile([P, 512], F32, name="psS", tag="psS")
                    nc.tensor.matmul(
                        ps[:, :nv],
                        lhsT=kT[:, h, j * P : (j + 1) * P],
                        rhs=qT[:, h, j * P :],
                        start=True,
                        stop=False,
                    )
                    # additive causal mask on the diagonal block (PE matmul)
                    nc.tensor.matmul(
                        ps[:, :P], lhsT=maskA[:], rhs=maskB[:],
                        start=False, stop=True,
                    )
                    nc.scalar.activation(
                        AT[:, j, j * P :], ps[:, :nv], ACTF.Exp,
                        bias=expb_sb[:], scale=SCALE,
                    )
                # per-tq row sums of the scaled probs via ones-column MMs
                for c in range(TC):
                    prs = psT.tile([P, P], F32, name="prs", tag="ptr")
                    for j in range(c + 1):
                        nc.tensor.matmul(
                            prs[:, :1],
                            lhsT=AT[:, j, c * P : (c + 1) * P],
                            rhs=ones_col_bf[:],
                            start=(j == 0),
                            stop=(j == c),
                        )
                    nc.vector.reciprocal(rinv_all[:, h, c : c + 1], prs[:, :1])
                py = psA.tile([P, 512], F32, name="psY", tag="psA")
                for j in range(TC):
                    nc.tensor.matmul(
                        py[:, j * P :],
                        lhsT=v_sb[:, j, h * HD : (h + 1) * HD],
                        rhs=AT[:, j, j * P :],
                        start=(j == 0),
                        stop=(j == TC - 1),
                    )
                nc.scalar.copy(yT[:, h, :], py[:])

            if l == 0 and srank >= 3:
                compute_mem_and_ca()
            # prefetch next layer's SA weights during this layer's tail
            if l + 1 < L:
                sal_next = wpool.tile(
                    [P, 3, 2, KC, W], F8, name="sa_sb", tag="sa_sb"
                )
                for mi in range(3):
                    nc.sync.dma_start(sal_next[:, mi], sa_d[l + 1, :, mi])
                sao_next = wpool.tile(
                    [P, KC, W], BF16, name="sao_sb", tag="sao_sb"
                )
                nc.sync.dma_start(sao_next[:], sao_d[l + 1])

            # per-head out projection (bf16); normalization folded into the
            # per-partition rinv of the fused residual accumulate.  Heads 0-1
            # accumulate via DVE stt; heads 2-3 offload to Act (rinv scale on
            # the copy) + Pool (SBUF-only residual add) to unload DVE.
            for c in range(TC):
                for h in range(H):
                    po = psT.tile([P, 512], F32, name="po", tag="ptr")
                    nc.tensor.matmul(
                        po[:],
                        lhsT=yT[:, h, c * P : (c + 1) * P],
                        rhs=saol_sb[:, h, :],
                        start=True,
                        stop=True,
                    )
                    base = x0_sb if (l == 0 and h == 0) else x_sb
                    sx = None
                    if h == H - 1 and ln_trivial:
                        sx = hot.tile([P, 1], F32, name="lnsx", tag="lnsx")
                    nc.vector.scalar_tensor_tensor(
                        x_sb[:, c, :], po[:], rinv_all[:, h, c : c + 1],
                        base[:, c, :], op0=ALU.mult, op1=ALU.add,
                        accum_out=(sx[:] if sx is not None else None),
                    )
                if not row_biases_zero:
                    pob = psS.tile([P, 512], F32, name="psS", tag="psS")
                    nc.tensor.matmul(
                        pob[:], lhsT=ones_bf[:], rhs=sabo_sb[l][:],
                        start=True, stop=True,
                    )
                    nc.vector.tensor_add(x_sb[:, c, :], x_sb[:, c, :], pob[:])
                if srank > 3:
                    # fused per-chunk ln1 -> +ca -> ln2 (other engines run
                    # these while PE streams the next chunk's matmuls)
                    if ln_trivial:
                        ln_chunk_fast(c, sx)
                    else:
                        ln_chunk(0, l, c)
                    nc.gpsimd.tensor_add(
                        x_sb[:, c, :], x_sb[:, c, :], ca_bc[l][:]
                    )
                    ln_chunk(1, l, c)
            if srank == 2:
                layer_norm(0, l)
                break
            if srank == 3:
                for c in range(TC):
                    ln_chunk(0, l, c)
                    nc.gpsimd.tensor_add(
                        x_sb[:, c, :], x_sb[:, c, :], ca_bc[l][:]
                    )
                    ln_chunk(1, l, c)
                break

            # ---- ffn (x single-fp8: 2-term ffn1, no lo-split ops) ----
            xT2h = work.tile([P, KC, T], F8, name="xT2h", tag="xT2h")
            transpose_x_to(xT2h)
            ff1_hi = ffl_sb[:, 0, 0].rearrange("p (k f) -> p k f", k=KC)
            ff1_lo = ffl_sb[:, 0, 1].rearrange("p (k f) -> p k f", k=KC)
            ff2_hi = ffl_sb[:, 1, 0].rearrange("p (m w) -> p m w", m=FFC)
            ff2_lo = ffl_sb[:, 1, 1].rearrange("p (m w) -> p m w", m=FFC)

            hT = work.tile([P, FFC, T], F8, name="hT", tag="hT")
            for m in range(FFC):
                ph = psA.tile([P, 512], F32, name="psA", tag="psA")
                first = True
                # 2 terms: x8@w_hi, x8@w_lo (kt-paired DR)
                for fw in (ff1_hi, ff1_lo):
                    for k0 in range(0, KC, 2):
                        nc.tensor.matmul(
                            ph[:],
                            lhsT=fw[:, k0 : k0 + 2, m * P : (m + 1) * P],
                            rhs=xT2h[:, k0 : k0 + 2, :],
                            start=first,
                            stop=(fw is ff1_lo and k0 == KC - 2),
                            perf_mode=DR,
                        )
                        first = False
                nc.scalar.activation(
                    hT[:, m, :], ph[:], ACTF.Relu,
                    bias=ff1b_sb[l][:, m : m + 1], scale=H_OSCALE,
                )
            for c in range(TC):
                pf2 = psA.tile([P, 512], F32, name="psA", tag="psA")
                first = True
                if not row_biases_zero:
                    nc.tensor.matmul(
                        pf2[:], lhsT=ones_bf[:], rhs=ff2b_sb[l][:],
                        start=True, stop=False,
                    )
                    first = False
                for hl in range(2):
                    for m0 in range(0, FFC, 2):
                        nc.tensor.matmul(
                            pf2[:],
                            lhsT=hT[:, m0 : m0 + 2, c * P : (c + 1) * P],
                            rhs=(ff2_hi if hl == 0 else ff2_lo)[:, m0 : m0 + 2, :],
                            start=first,
                            stop=(hl == 1 and m0 == FFC - 2),
                            perf_mode=DR,
                        )
                        first = False
                nc.vector.scalar_tensor_tensor(
                    x_sb[:, c, :], pf2[:], F2_OSCALE, x_sb[:, c, :],
                    op0=ALU.mult, op1=ALU.add,
                )
                ln_chunk(2, l, c)
            if srank == 4:
                break

        # ================= logits =================
        xTfh = work.tile([P, KC, T], F8, name="xTfh", tag="xT2h")
        xTfl = work.tile([P, KC, T], F8, name="xTfl", tag="xT2l")
        if srank >= 5:
            transpose_x_to(xTfh, xTfl)

        _nvg = NVG if srank >= 99 else (1 if srank >= 6 else 0)
        # cycle freed weight-tag buffers into the outw stream: 5 dedicated
        # bufs + 4 reclaimed tags = 9 groups in flight (deeper prefetch)
        _extra_tags = ["ca_sb", "sa_sb", "sao_sb", "ff_sb"]  # r = 6..8, 11
        for vg in range(_nvg):
            r = vg % 12
            if r < 5:
                wlog = wlogp.tile([P, 2, KC, VG], F8, name="wlog", tag="wlog")
            elif r in (5, 9, 10):
                # reclaimed work-pool buffers: visw frees after the memory
                # vector (~25us), hT/x0 free after layer 1
                tg = {5: "visw", 9: "hT", 10: "x0"}[r]
                wlog = work.tile([P, 2, KC, VG], F8, name="wlog", tag=tg)
            else:
                tg = {6: "ca_sb", 7: "sa_sb", 8: "sao_sb", 11: "ff_sb"}[r]
                wlog = wpool.tile([P, 2, KC, VG], F8, name="wlog", tag=tg)
            nc.sync.dma_start(wlog[:], outw_d[vg])
            w_hi, w_lo = wlog[:, 0], wlog[:, 1]
            for c in range(TC):
                # per-chunk output tile + write: the 8-deep quarter-group
                # ring frees copy buffers 4x finer than whole-group tiles
                ost = ostp.tile([P, VG], BF16, name="ost", tag="ost")
                for sv in range(NSV):
                    i3 = (c * NSV + sv) % 3
                    plp = (psA, psS, psT)[i3]
                    pl = plp.tile(
                        [P, 512], F32, name="psL",
                        tag=("psA", "psS", "ptr")[i3],
                    )
                    first = True
                    if not row_biases_zero:
                        nc.tensor.matmul(
                            pl[:, :SV],
                            lhsT=ones_bf[:],
                            rhs=outb_sb[:, vg * VG + sv * SV : vg * VG + (sv + 1) * SV],
                            start=True,
                            stop=False,
                        )
                        first = False
                    for xt, wt in ((xTfh, w_hi), (xTfl, w_hi), (xTfh, w_lo)):
                        for k0 in range(0, KC, 2):
                            nc.tensor.matmul(
                                pl[:, :SV],
                                lhsT=xt[:, k0 : k0 + 2, c * P : (c + 1) * P],
                                rhs=wt[:, k0 : k0 + 2, sv * SV : (sv + 1) * SV],
                                start=first,
                                stop=(xt is xTfh and wt is w_lo and k0 == KC - 2),
                                perf_mode=DR,
                            )
                            first = False
                    copy_scaled(ost[:, sv * SV : (sv + 1) * SV],
                                pl[:, :SV], LG_OSCALE)
                # per-chunk write on the idle Pool SWDGE queue (stalled SP
                # reads can't head-of-line block it; Act.SEQ stays free)
                nc.gpsimd.dma_start(
                    out_d[:, c, vg * VG : (vg + 1) * VG], ost[:]
                )

        if stop_after is not None:
            xdbg = nc.dram_tensor(
                "xdbg", [P, TC, W], F32, kind="ExternalOutput"
            ).ap()
            nc.sync.dma_start(xdbg[:], x_sb[:])

    nc.compile()
    return nc


_BUILD_CACHE = {}


def _get_nc(row_biases_zero, ln_trivial, cbz=True):
    key = (row_biases_zero, ln_trivial, cbz)
    if key not in _BUILD_CACHE:
        _BUILD_CACHE[key] = _build(*key)
    return _BUILD_CACHE[key]


def _prep_in_maps(inputs):
    f32 = np.float32
    features = np.asarray(inputs["features"], f32)          # (N, D)
    captions = np.asarray(inputs["captions"])               # (N, T) int
    emb = np.asarray(inputs["emb"], f32)                    # (V, W)
    pe = np.asarray(inputs["pe"], f32)                      # (T, W)

    row_biases_zero = all(
        not np.any(np.asarray(inputs[k]))
        for k in ("sa_bv", "sa_bo", "ff2_b", "out_b")
    )
    ln_trivial = all(
        np.all(np.asarray(inputs[f"ln{i}_w"]) == 1.0)
        and not np.any(np.asarray(inputs[f"ln{i}_b"]))
        for i in (1, 2, 3)
    )


    # sa weights: [L, P, 3(q,k,v), 2(hi/lo), KC, W] fp8 at scale SW
    sa_pack = np.empty((L, P, 3, 2, KC, W), F8_NP)
    for mi, k in enumerate(("sa_wq", "sa_wk", "sa_wv")):
        wrapped = _wrap_p(np.asarray(inputs[k], f32), f32)  # [L, P, KC, W]
        hi, lo = _split_f8(wrapped, SW)
        sa_pack[:, :, mi, 0] = hi
        sa_pack[:, :, mi, 1] = lo
    sao_pack = _wrap_p(np.asarray(inputs["sa_wo"]), BF16_NP)  # [L, P, KC, W]
    ca_pack = np.stack(
        [_wrap_p(np.asarray(inputs[k]), BF16_NP) for k in ("ca_wv", "ca_wo")],
        axis=1,
    )
    ca_pack = np.ascontiguousarray(np.moveaxis(ca_pack, 1, 2))
    # ff: [L, P, 2(f1,f2), 2(hi/lo), KC*FF] fp8
    ff_pack = np.empty((L, P, 2, 2, KC * FF), F8_NP)
    f1w = _wrap_p(np.asarray(inputs["ff1_w"], f32), f32).reshape(L, P, KC * FF)
    f2w = _wrap_p(np.asarray(inputs["ff2_w"], f32), f32).reshape(L, P, FFC * W)
    for fi, wv in enumerate((f1w, f2w)):
        hi, lo = _split_f8(wv, SW)
        ff_pack[:, :, fi, 0] = hi
        ff_pack[:, :, fi, 1] = lo
    # outw: [NVG, P, 2(hi/lo), KC, VG] fp8
    oww = np.moveaxis(
        np.asarray(inputs["out_w"], f32).reshape(KC, P, V), 0, 1
    )  # [P, KC, V]
    ow_hi, ow_lo = _split_f8(oww, SW)
    outw_pack = np.empty((NVG, P, 2, KC, VG), F8_NP)
    for g in range(NVG):
        outw_pack[g, :, 0] = ow_hi[:, :, g * VG : (g + 1) * VG]
        outw_pack[g, :, 1] = ow_lo[:, :, g * VG : (g + 1) * VG]

    cpack = np.zeros((P, CPACK_COLS), f32)
    o = 0
    cpack[:, o : o + KC] = _wrap_vec(np.asarray(inputs["vis_b"]), f32); o += KC
    feat_off = o; o += DC  # per-core features slot
    sabq = _wrap_vec(np.asarray(inputs["sa_bq"]), f32)
    sabk = _wrap_vec(np.asarray(inputs["sa_bk"]), f32)
    cabv = _wrap_vec(np.asarray(inputs["ca_bv"]), f32)
    ff1b = _wrap_vec(np.asarray(inputs["ff1_b"]), f32) * SH  # pre-scaled
    cabo = np.asarray(inputs["ca_bo"], f32)
    for l in range(L):
        cpack[:, o + 4 * l : o + 4 * (l + 1)] = sabq[l]
    o += 4 * L
    for l in range(L):
        cpack[:, o + 4 * l : o + 4 * (l + 1)] = sabk[l]
    o += 4 * L
    for l in range(L):
        cpack[:, o + 4 * l : o + 4 * (l + 1)] = cabv[l]
    o += 4 * L
    for l in range(L):
        cpack[:, o + FFC * l : o + FFC * (l + 1)] = ff1b[l]
    o += FFC * L
    for l in range(L):
        cpack[0, o + W * l : o + W * (l + 1)] = cabo[l]
    o += W * L
    assert o == CPACK_COLS

    shared = {
        "visw": _wrap_p(np.asarray(inputs["vis_w"]), BF16_NP),
        "sa": np.ascontiguousarray(sa_pack),
        "sao": sao_pack,
        "ca": ca_pack,
        "ff": np.ascontiguousarray(ff_pack),
        "outw": np.ascontiguousarray(outw_pack),
    }
    if not row_biases_zero:
        shared["sabv"] = np.ascontiguousarray(
            (np.asarray(inputs["sa_bv"], f32) * (SX * SW))
            .astype(BF16_NP).reshape(L, 1, W)
        )
        shared["sabo"] = np.ascontiguousarray(
            np.asarray(inputs["sa_bo"]).astype(BF16_NP).reshape(L, 1, W)
        )
        shared["ff2b"] = np.ascontiguousarray(
            (np.asarray(inputs["ff2_b"], f32) * (SH * SW))
            .astype(BF16_NP).reshape(L, 1, W)
        )
        shared["outb"] = np.ascontiguousarray(
            (np.asarray(inputs["out_b"], f32) * (SX * SW))
            .astype(BF16_NP).reshape(1, V)
        )
    if not ln_trivial:
        for i in (1, 2, 3):
            shared[f"ln{i}w"] = np.ascontiguousarray(
                np.asarray(inputs[f"ln{i}_w"], f32).reshape(L, 1, W)
            )
            shared[f"ln{i}b"] = np.ascontiguousarray(
                np.asarray(inputs[f"ln{i}_b"], f32).reshape(L, 1, W)
            )

    in_maps = []
    for i in range(N):
        m = dict(shared)
        x = emb[np.asarray(captions[i])] + pe  # [T, W] f32
        m["x0"] = np.ascontiguousarray(
            x.reshape(TC, P, W).transpose(1, 0, 2).astype(BF16_NP)
        )
        m["xt0"] = np.ascontiguousarray(
            (x * SX).T.reshape(KC, P, T).transpose(1, 0, 2).astype(F8_NP)
        )
        cp = cpack.copy()
        cp[:, feat_off : feat_off + DC] = features[i].reshape(DC, P).T
        m["cpack"] = cp
        in_maps.append(m)
    return in_maps, row_biases_zero, ln_trivial


def kernel(**inputs) -> np.ndarray:
    in_maps, row_biases_zero, ln_trivial = _prep_in_maps(inputs)
    cbz = all(
        not np.any(np.asarray(inputs[k])) for k in ("sa_bq", "sa_bk", "ff1_b")
    )
    nc = _get_nc(row_biases_zero, ln_trivial, cbz)
    # The axon/NRT path occasionally throws a transient
    # NRT_EXEC_UNIT_UNRECOVERABLE on dispatch; the devices recover, so retry.
    last_err = None
    for attempt in range(3):
        try:
            res = run_bass_kernel_spmd(nc, in_maps, core_ids=list(range(N)))
            break
        except Exception as e:  # noqa: BLE001
            last_err = e
            import time as _time

            _time.sleep(5.0)
    else:
        raise last_err
    out = np.empty((N, T, V), np.float32)
    for i in range(N):
        lg = np.asarray(res.results[i]["logits"])  # [P, TC, V] bf16
        out[i] = np.moveaxis(lg.astype(np.float32), 0, 1).reshape(T, V)
    return out
